# revision 1
# baseline (speedup 1.0000x reference)
"""DAGCN kernel v3 for Trainium2, 8 NeuronCores, sharded over T (3 t/core).

Math per t (N=512 nodes, C=O=64, B=32, K=3):
  A   = relu(E E^T)                 (symmetric)
  PU  = exp(A)                      (symmetric; exp(relu(x)) bounded by e^~16)
  inv[n] = 1/sum_s PU[n,s];  S = diag(inv) PU  (softmax scores)
  d[n] = S[n,n];  xg1 = S @ x;  xg2 = 2 d * xg1 - x
  out  = x@W0 + xg1@W1 + xg2@W2 + bias      (per-n [C,O] weights)

Key tricks:
  - Transposed-orientation message passing: rhs1[m,n] = inv[n]*PU[n,m]
    equals row-scaled Sr[a,b]=inv[a]*PU[a,b] transposed (PU symmetric), so
    scale-by-inv happens with n on partitions (per-partition scalar), then a
    cheap 512x512 on-chip transpose. e1T matmuls then yield xg1^T directly
    in [c, b, n] stack layout -- no per-b transposes at all.
  - Identity fold: rhs2[m,n] = (2 d inv)[n]*PU[n,m] - I[m,n] makes the same
    pass produce xg2^T without a separate subtract (diag blocks of the
    transposed score tile get ident subtracted once).
  - Host relayout (pure transpose/cast, no math): weights stacked as
    wst1=[W0;W1] -> [T,128,N,O] bf16 and wst2=[W2;bias] -> [T,65,N,O] bf16,
    x in both [T,N,B,C] and [T,C,B,N] bf16. All DMAs run at >=4KB
    contiguous runs (~343 GB/s).
  - bias folded as stationary ones-row (row 64 of stack2) x wst2 row 64.
  - Per-n second contraction: stationary = stack slices [128|65, 32b],
    moving = weight slices [.,64o], accumulating psum [32,512] per 8 nodes;
    output lands as [b, n, o] -- contiguous store, no output transposes.
"""
import sys

sys.path.insert(0, "/opt/trn_rl_repo")
import numpy as np
CFG = "oc_dap pst1 ps4 on_pool sr2_dve"


B, T, N, C, O, K = 32, 24, 512, 64, 64, 3
NCORES = 8
T_LOC = T // NCORES  # 3 time steps per core

_CACHE = {}


def build_bass(reps=1):
    if ("nc", reps) in _CACHE:
        return _CACHE[("nc", reps)]
    from contextlib import ExitStack

    import concourse.mybir as mybir
    from concourse import bacc
    import concourse.tile as tile
    from concourse.bass import ts
    from concourse.masks import make_identity

    f32 = mybir.dt.float32
    f32r = mybir.dt.float32r
    bf16 = mybir.dt.bfloat16
    Alu = mybir.AluOpType
    Act = mybir.ActivationFunctionType
    AX = mybir.AxisListType.X

    nc = bacc.Bacc()
    xm_d = nc.dram_tensor("xmbc_sh", [T_LOC, N, B, C], bf16, kind="ExternalInput")
    xt_d = nc.dram_tensor("xT_sh", [T_LOC, C, B, N], bf16, kind="ExternalInput")
    e_d = nc.dram_tensor("emb_sh", [T_LOC, N], f32r, kind="ExternalInput")
    w1_d = nc.dram_tensor("wst1_sh", [T_LOC, 2 * C, N, O], bf16, kind="ExternalInput")
    w2_d = nc.dram_tensor("wst2_sh", [T_LOC, C + 1, N, O], bf16, kind="ExternalInput")
    on_d = nc.dram_tensor("ones_sh", [1, B * N], bf16, kind="ExternalInput")
    o_d = nc.dram_tensor("out_sh", [B, T_LOC, N, O], bf16, kind="ExternalOutput")

    with tile.TileContext(nc) as tc, ExitStack() as ctx:
        p1 = ctx.enter_context(tc.tile_pool(name="singles", bufs=1))
        p_sc = ctx.enter_context(tc.tile_pool(name="scores", bufs=2))
        p_er = ctx.enter_context(tc.tile_pool(name="erow", bufs=1))
        p_pu = ctx.enter_context(tc.tile_pool(name="pu", bufs=4))
        p_srt = ctx.enter_context(tc.tile_pool(name="srt", bufs=4))
        p_x = ctx.enter_context(tc.tile_pool(name="xin", bufs=4 if "xb4" in CFG else 5))
        p_w = ctx.enter_context(tc.tile_pool(name="wt", bufs=4))
        p_ob = ctx.enter_context(tc.tile_pool(name="ob", bufs=2))
        p_ps = ctx.enter_context(tc.tile_pool(name="ps", bufs=4 if "ps4" in CFG else 3, space="PSUM"))
        p_pso = ctx.enter_context(tc.tile_pool(name="pso", bufs=2 if "pso2" in CFG else 5 if "pso5" in CFG else 4 if "pso4" in CFG else 3, space="PSUM"))
        p_pst = None if "pstshare" in CFG else ctx.enter_context(
            tc.tile_pool(name="pst", bufs=1 if "pst1" in CFG else 2, space="PSUM"))

        identb = p1.tile([128, 128], bf16)
        make_identity(nc, identb)
        NWARM = 0
        if NWARM:
            jps = p_pst.tile([128, N], bf16, tag="tr")
            for _ in range(NWARM):
                nc.tensor.transpose(jps[:, 0:128], identb[:], identb[:])

        # persistent stacks; optionally double-buffered by t parity
        NSTK = 2 if "dbl" in CFG else 1
        stack1s = [p1.tile([128, B, N], bf16, name=f"stk1_{i}", tag=f"s1_{i}") for i in range(NSTK)]
        stack2s = [p1.tile([C + 1, B, N], bf16, name=f"stk2_{i}", tag=f"s2_{i}") for i in range(NSTK)]
        for s2 in stack2s:
            (nc.scalar if "on_act" in CFG else nc.gpsimd if "on_pool" in CFG else nc.sync).dma_start(
                out=s2[C:C + 1, :, :].rearrange("p b n -> p (b n)"), in_=on_d[:])

        from contextlib import nullcontext
        for tt in range(T_LOC * reps):
            t = tt % T_LOC
            stack1 = stack1s[t % NSTK]
            stack2 = stack2s[t % NSTK]
            # ---------------- scores phase ----------------
            e_row = p_er.tile([1, N], f32r, tag="erow")
            nc.sync.dma_start(out=e_row, in_=e_d[t][None, :])
            e_col = p_sc.tile([128, 4], f32r, tag="ecol")
            nc.sync.dma_start(out=e_col, in_=e_d[t].rearrange("(c p) -> p c", p=128))

            pus, invs, d2s = [], [], []
            for mc in range(4):
                ps = p_ps.tile([128, N], f32, tag="big")
                nc.tensor.matmul(ps[:], e_row[:, ts(mc, 128)], e_row[:],
                                 start=True, stop=True)
                pu = p_pu.tile([128, N], bf16, tag="pu")
                nc.scalar.activation(pu[:], ps[:], Act.Exp)
                nc.vector.tensor_single_scalar(pu[:], pu[:], 1.0, Alu.max)
                pus.append(pu)
            for mc in range(4):
                rs = p_sc.tile([128, 1], f32, tag="rs")
                if "red_pool" in CFG:
                    nc.gpsimd.tensor_reduce(rs[:], pus[mc][:], axis=AX, op=Alu.add)
                else:
                    nc.vector.tensor_reduce(rs[:], pus[mc][:], axis=AX, op=Alu.add)
                inv = p_sc.tile([128, 1], f32, tag="inv")
                nc.vector.reciprocal(inv[:], rs[:])
                sq = p_sc.tile([128, 1], f32, tag="sq")
                nc.vector.tensor_mul(sq[:], e_col[:, mc:mc + 1], e_col[:, mc:mc + 1])
                esq = p_sc.tile([128, 1], f32, tag="esq")
                nc.scalar.activation(esq[:], sq[:], Act.Exp)
                t1 = p_sc.tile([128, 1], f32, tag="t1")
                nc.vector.tensor_mul(t1[:], esq[:], inv[:])
                t2 = p_sc.tile([128, 1], f32, tag="t2")
                nc.vector.tensor_mul(t2[:], t1[:], inv[:])
                d2 = p_sc.tile([128, 1], f32, tag="d2")
                nc.vector.tensor_add(d2[:], t2[:], t2[:])  # 2*d*inv
                invs.append(inv)
                d2s.append(d2)

            # row-scaled score tiles (still [a, b] layout)
            srs, sr2s = [], []
            for mc in range(4):
                sr = p_pu.tile([128, N], bf16, tag="sr")
                if "sr_act" in CFG:
                    nc.scalar.activation(sr[:], pus[mc][:], Act.Copy,
                                         scale=invs[mc][:])
                else:
                    nc.vector.tensor_scalar_mul(sr[:], pus[mc][:], invs[mc][:])
                srs.append(sr)
                sr2 = p_pu.tile([128, N], bf16, tag="sr2")
                if "sr2_dve" in CFG:
                    nc.vector.tensor_scalar_mul(sr2[:], pus[mc][:], d2s[mc][:])
                else:
                    nc.gpsimd.tensor_scalar_mul(sr2[:], pus[mc][:], d2s[mc][:])
                sr2s.append(sr2)

            # transpose to rhs layout [m, n]: rhsT_mc[:, ac*128:] = sr_ac[:, mc]^T
            srts, sr2ts = [], []
            for mc in range(4):
                pst = (p_ps if "pstshare" in CFG else p_pst).tile(
                    [128, N], bf16, tag="big" if "pstshare" in CFG else "tr")
                for ac in range(4):
                    nc.tensor.transpose(pst[:, ts(ac, 128)],
                                        srs[ac][:, ts(mc, 128)], identb[:])
                srt = p_srt.tile([128, N], bf16, tag="srt")
                nc.scalar.copy(out=srt[:], in_=pst[:])
                srts.append(srt)

                pst2 = (p_ps if "pstshare" in CFG else p_pst).tile(
                    [128, N], bf16, tag="big" if "pstshare" in CFG else "tr")
                for ac in range(4):
                    nc.tensor.transpose(pst2[:, ts(ac, 128)],
                                        sr2s[ac][:, ts(mc, 128)], identb[:])
                srt2 = p_srt.tile([128, N], bf16, tag="srt2")
                nc.vector.tensor_copy(out=srt2[:], in_=pst2[:])
                # subtract identity on the diagonal block (rhs2 = scaled - I)
                nc.vector.tensor_tensor(
                    out=srt2[:, ts(mc, 128)], in0=srt2[:, ts(mc, 128)],
                    in1=identb[:], op=Alu.subtract)
                sr2ts.append(srt2)

            # ---------------- x loads ----------------
            xms = []
            for mc in range(4):
                xm = p_x.tile([128, B, C], bf16, tag="xm")
                (nc.scalar if "xm_act" in CFG else nc.gpsimd if "xm_pool" in CFG else nc.sync).dma_start(out=xm, in_=xm_d[t, ts(mc, 128)])
                xms.append(xm)
            # x^T directly into stack1 rows 0-63
            if "xt2" in CFG:
                nc.sync.dma_start(out=stack1[0:C // 2, :, :], in_=xt_d[t, 0:C // 2])
                nc.sync.dma_start(
                    out=stack1[C // 2:C, :, :], in_=xt_d[t, C // 2:C])
            else:
                (nc.scalar if "xt_act" in CFG else nc.sync).dma_start(
                    out=stack1[0:C, :, :], in_=xt_d[t])

            # ---------------- e1T: build xg1^T, xg2^T stacks ----------------
            for pr in range(16):  # b-pairs
                b0 = 2 * pr
                ps1 = p_ps.tile([128, N], f32, tag="big")
                ps2 = p_ps.tile([128, N], f32, tag="big")
                for mc in range(4):
                    lhs = xms[mc][:, b0:b0 + 2, :].rearrange("p b c -> p (b c)")
                    nc.tensor.matmul(ps1[:], lhs, srts[mc][:],
                                     start=(mc == 0), stop=(mc == 3))
                for mc in range(4):
                    lhs = xms[mc][:, b0:b0 + 2, :].rearrange("p b c -> p (b c)")
                    nc.tensor.matmul(ps2[:], lhs, sr2ts[mc][:],
                                     start=(mc == 0), stop=(mc == 3))
                def _cp(eng, out, in_):
                    if eng is nc.scalar:
                        eng.copy(out=out, in_=in_)
                    else:
                        eng.tensor_copy(out=out, in_=in_)
                if "ec_rot3" in CFG:
                    engs = [nc.vector, nc.scalar, nc.gpsimd]
                    k0 = pr % 3
                    _cp(engs[k0], stack1[C:, b0, :], ps1[0:C])
                    _cp(engs[(k0 + 1) % 3], stack1[C:, b0 + 1, :], ps1[C:])
                    _cp(engs[(k0 + 2) % 3], stack2[0:C, b0, :], ps2[0:C])
                    _cp(engs[k0], stack2[0:C, b0 + 1, :], ps2[C:])
                elif "ec_act3" in CFG:
                    _cp(nc.vector, stack1[C:, b0, :], ps1[0:C])
                    _cp(nc.scalar, stack1[C:, b0 + 1, :], ps1[C:])
                    _cp(nc.scalar, stack2[0:C, b0, :], ps2[0:C])
                    _cp([nc.vector, nc.scalar][pr % 2], stack2[0:C, b0 + 1, :], ps2[C:])
                else:
                    engs = [nc.vector, nc.scalar]
                    _cp(engs[pr % 2], stack1[C:, b0, :], ps1[0:C])
                    _cp(engs[(pr + 1) % 2], stack1[C:, b0 + 1, :], ps1[C:])
                    _cp(engs[pr % 2], stack2[0:C, b0, :], ps2[0:C])
                    _cp(engs[(pr + 1) % 2], stack2[0:C, b0 + 1, :], ps2[C:])

            # ---------------- per-n contraction ----------------
            OBN = 32 if "ob32" in CFG else 64
            WN = 32 if "wn32" in CFG else 64
            for hc in range(8):  # 64-n chunks
                w1ts, w2ts = [], []
                for wc in range(64 // WN):
                    w1t = p_w.tile([2 * C, WN, O], bf16, tag="w1", name=f"w1t_{t}_{hc}_{wc}")
                    w2t = p_w.tile([C + 1, WN, O], bf16, tag="w2", name=f"w2t_{t}_{hc}_{wc}")
                    with nullcontext():
                        (nc.scalar if "w1_act" in CFG else nc.gpsimd).dma_start(
                            out=w1t, in_=w1_d[t, :, hc * 64 + wc * WN: hc * 64 + (wc + 1) * WN, :])
                        (nc.sync if "w2_sp" in CFG else nc.gpsimd).dma_start(
                            out=w2t, in_=w2_d[t, :, hc * 64 + wc * WN: hc * 64 + (wc + 1) * WN, :])
                    w1ts.append(w1t)
                    w2ts.append(w2t)
                PG = 16 if "pg16" in CFG else 8
                for ho in range(64 // OBN):
                    out_sb = p_ob.tile([B, OBN, O], bf16, tag="osb")
                    for g in range(OBN // PG):  # PG-n psum groups
                        n0 = ho * OBN + g * PG
                        ps_o = p_pso.tile([B, PG, O], f32, tag="po")
                        for j in range(PG):
                            nl = n0 + j
                            ng = hc * 64 + nl
                            nc.tensor.matmul(
                                ps_o[:, j, :], stack1[:, :, ng],
                                w1ts[nl // WN][:, nl % WN, :], start=True, stop=False)
                            nc.tensor.matmul(
                                ps_o[:, j, :], stack2[:, :, ng],
                                w2ts[nl // WN][:, nl % WN, :], start=False, stop=True)
                        dst = out_sb[:, g * PG:(g + 1) * PG, :].rearrange(
                            "b n o -> b (n o)")
                        srcp = ps_o[:].rearrange("b n o -> b (n o)")
                        if "oc_dap" in CFG:
                            oeng = [nc.vector, nc.scalar][g % 2]
                        elif "oc_rot3" in CFG:
                            oeng = [nc.vector, nc.scalar, nc.gpsimd][g % 3]
                        else:
                            oeng = nc.gpsimd
                        if oeng is nc.scalar:
                            oeng.copy(out=dst, in_=srcp)
                        else:
                            oeng.tensor_copy(out=dst, in_=srcp)
                    nc.sync.dma_start(
                        out=o_d[:, t, hc * 64 + ho * OBN: hc * 64 + (ho + 1) * OBN,
                                :], in_=out_sb[:])

    nc.finalize()
    _CACHE[("nc", reps)] = nc
    return nc


def make_in_maps(inputs):
    import ml_dtypes
    bf16 = ml_dtypes.bfloat16

    x = np.asarray(inputs["x"], dtype=np.float32)
    emb = np.asarray(inputs["dn_embeddings"], dtype=np.float32)
    w = np.asarray(inputs["weights_pool"], dtype=np.float32)
    bias = np.asarray(inputs["bias_pool"], dtype=np.float32)

    in_maps = []
    for c in range(NCORES):
        sl = slice(c * T_LOC, (c + 1) * T_LOC)
        xs = x[:, sl]  # [B, T_LOC, N, C]
        ws = w[sl]  # [T_LOC, N, K, C, O]
        wst1 = np.ascontiguousarray(
            ws[:, :, 0:2].transpose(0, 2, 3, 1, 4)).reshape(T_LOC, 2 * C, N, O)
        wst2 = np.empty((T_LOC, C + 1, N, O), np.float32)
        wst2[:, 0:C] = ws[:, :, 2].transpose(0, 2, 1, 3)
        wst2[:, C] = bias[sl]
        in_maps.append({
            "xmbc_sh": np.ascontiguousarray(
                xs.transpose(1, 2, 0, 3)).astype(bf16),
            "xT_sh": np.ascontiguousarray(
                xs.transpose(1, 3, 0, 2)).astype(bf16),
            "emb_sh": np.ascontiguousarray(emb[sl]),
            "wst1_sh": np.ascontiguousarray(wst1).astype(bf16),
            "wst2_sh": np.ascontiguousarray(wst2).astype(bf16),
            "ones_sh": np.ones((1, B * N), dtype=bf16),
        })
    return in_maps


def run_spmd(inputs, **kwargs):
    from concourse.bass_utils import run_bass_kernel_spmd

    nc = build_bass()
    in_maps = make_in_maps(inputs)
    res = run_bass_kernel_spmd(nc, in_maps, core_ids=list(range(NCORES)), **kwargs)
    out = np.concatenate([r["out_sh"] for r in res.results], axis=1)
    return out.astype(np.float32), res


def kernel(**inputs):
    out, _ = run_spmd(inputs)
    return out



# revision 9
# speedup vs baseline: 1.1094x; 1.1094x over previous
"""DAGCN kernel v5 for Trainium2, 8 NeuronCores, sharded over T (3 t/core).

Math per t (N=512 nodes, C=O=64, B=32, K=3):
  A    = relu(E E^T)  (rank-1 outer product, symmetric)
  PU   = exp(A) = max(exp(e_n e_m), 1)   (symmetric)
  Z_n  = sum_m PU[m, n]  (column sums == row sums by symmetry)
  S    = PU / Z  (row softmax);  d_n = S[n,n] = exp(e_n^2)/Z_n
  xg1  = S @ x;  xg2 = 2 d xg1 - x
  out  = x W0 + xg1 W1 + xg2 W2 + bias
       = x (W0 - W2) + xg1 W1 + (2 d xg1) W2 + bias     <- regrouped

Key structure vs the v3 baseline (185us):
  - Algebraic regroup removes the second message pass entirely: only
    xg1 = S@x is computed on the PE (64 vs 128 big matmuls per t), and
    xg1d = 2d*xg1 is a cheap elementwise multiply of the same psum.
  - No PE transposes for scores: PU is symmetric, so the e1T moving
    operand srt[m,n] = PU[m,n] * inv[n] is a column-scale of the PU tile.
    Column broadcasts (inv, 2d) are materialized as [128, N] tiles via a
    k=1 ones-row matmul (the PE is the only partition broadcaster).
  - Column sums via k=1 matmul with a ones column (PE, not DVE).
  - Weights quantized by numeric class: M2 weights [W1; W2] ship as
    fp8 e3m4 (they only multiply the small xg1/xg1d terms; measured
    absmax-rel ~1.3e-2 total), while [W0-W2; bias] stays bf16 (it
    multiplies x, 99.5% of output variance). Weight DMA drops from
    12.06 to 8.06 MiB/t. W0-W2 is precombined on the host.
  - Final contraction per n: 2 matmuls (k=65 bf16, k=128 with fp8
    moving), psum [32b, 8n, 64o] per group, contiguous [b, n, o] store.
"""
import sys

sys.path.insert(0, "/opt/trn_rl_repo")
import numpy as np

CFG = ""

B, T, N, C, O, K = 32, 24, 512, 64, 64, 3
NCORES = 8
T_LOC = T // NCORES  # 3 time steps per core

_CACHE = {}


def build_bass(reps=1):
    if ("nc", reps) in _CACHE:
        return _CACHE[("nc", reps)]
    from contextlib import ExitStack

    import concourse.mybir as mybir
    from concourse import bacc
    import concourse.tile as tile
    from concourse.bass import ts

    f32 = mybir.dt.float32
    f32r = mybir.dt.float32r
    bf16 = mybir.dt.bfloat16
    f8e3 = mybir.dt.float8e3
    Alu = mybir.AluOpType
    Act = mybir.ActivationFunctionType

    nc = bacc.Bacc()
    xm_d = nc.dram_tensor("xmbc_sh", [T_LOC, N, B, C], bf16, kind="ExternalInput")
    xt_d = nc.dram_tensor("xT_sh", [T_LOC, C, B, N], bf16, kind="ExternalInput")
    e_d = nc.dram_tensor("emb_sh", [T_LOC, N], f32r, kind="ExternalInput")
    w0b_d = nc.dram_tensor("w0b_sh", [T_LOC, C + 1, N, O], bf16, kind="ExternalInput")
    w12_d = nc.dram_tensor("w12_sh", [T_LOC, 2 * C, N, O], f8e3, kind="ExternalInput")
    on_d = nc.dram_tensor("ones_sh", [1, B * N], bf16, kind="ExternalInput")
    o_d = nc.dram_tensor("out_sh", [B, T_LOC, N, O], bf16, kind="ExternalOutput")

    with tile.TileContext(nc) as tc, ExitStack() as ctx:
        p1 = ctx.enter_context(tc.tile_pool(name="singles", bufs=1))
        p_row = ctx.enter_context(tc.tile_pool(name="rows", bufs=2))
        p_bc = ctx.enter_context(tc.tile_pool(name="bcast", bufs=2))
        p_pu = ctx.enter_context(tc.tile_pool(name="pu", bufs=4))
        p_srt = ctx.enter_context(tc.tile_pool(name="srt", bufs=4))
        p_x = ctx.enter_context(tc.tile_pool(name="xin", bufs=5))
        p_w0 = ctx.enter_context(tc.tile_pool(name="w0", bufs=4))
        p_w12 = ctx.enter_context(tc.tile_pool(name="w12", bufs=4))
        p_ob = ctx.enter_context(tc.tile_pool(name="ob", bufs=2))
        p_ps = ctx.enter_context(tc.tile_pool(name="ps", bufs=4, space="PSUM"))
        p_pso = ctx.enter_context(tc.tile_pool(name="pso", bufs=3, space="PSUM"))

        # persistent stacks:
        #   stack1 [65, B, N]  rows 0:64 x^T (c,b,n), row 64 ones (bias lane)
        #   stack2 [128, B, N] rows 0:64 xg1^T, rows 64:128 xg1d^T
        stack1 = p1.tile([C + 1, B, N], bf16, name="stk1", tag="s1")
        stack2 = p1.tile([2 * C, B, N], bf16, name="stk2", tag="s2")
        nc.gpsimd.dma_start(
            out=stack1[C:C + 1, :, :].rearrange("p b n -> p (b n)"), in_=on_d[:])
        # ones column [128, 1] for column-sum matmuls (k=1 trick needs a
        # [1, 128] stationary; ones row slice of on_d serves both)
        ones_col = p1.tile([128, 1], bf16, name="onec", tag="oc")
        nc.gpsimd.dma_start(out=ones_col, in_=on_d[0, 0:128].rearrange("(p f) -> p f", f=1))
        ones_row = p1.tile([1, 128], bf16, name="oner", tag="or")
        nc.gpsimd.dma_start(out=ones_row, in_=on_d[:, 0:128])

        for tt in range(T_LOC * reps):
            t = tt % T_LOC
            # ---------------- scores phase ----------------
            e_row = p_row.tile([1, N], f32r, tag="erow")
            nc.sync.dma_start(out=e_row, in_=e_d[t][None, :])

            # x loads early so DMA overlaps scores compute
            xms = []
            for mc in range(4):
                xm = p_x.tile([128, B, C], bf16, tag="xm")
                nc.sync.dma_start(out=xm, in_=xm_d[t, ts(mc, 128)])
                xms.append(xm)
            nc.sync.dma_start(out=stack1[0:C, :, :], in_=xt_d[t])

            # PU tiles: pu_mc[m, n] = max(exp(e_m * e_n), 1)  (4 m-chunks)
            pus = []
            for mc in range(4):
                ps = p_ps.tile([128, N], f32, tag="big")
                nc.tensor.matmul(ps[:], e_row[:, ts(mc, 128)], e_row[:],
                                 start=True, stop=True)
                pu = p_pu.tile([128, N], bf16, tag="pu")
                nc.scalar.activation(pu[:], ps[:], Act.Exp)
                nc.vector.tensor_single_scalar(pu[:], pu[:], 1.0, Alu.max)
                pus.append(pu)

            # column sums Z[1, n] = sum_m PU[m, n] via k=1 ones matmuls
            zs_ps = p_ps.tile([128, N], f32, tag="big")
            for mc in range(4):
                nc.tensor.matmul(zs_ps[0:1, :], ones_col[:], pus[mc][:],
                                 start=(mc == 0), stop=(mc == 3))
            inv_row = p_row.tile([1, N], bf16, tag="invr")
            with nc.allow_low_precision(reason="inv feeds bf16 bcast matmul"):
                nc.vector.reciprocal(inv_row[:], zs_ps[0:1, :])
            # d2_row = 2 * exp(e^2) * inv
            sq = p_row.tile([1, N], f32, tag="sq")
            nc.vector.tensor_mul(sq[:], e_row[:], e_row[:])
            esq = p_row.tile([1, N], f32, tag="esq")
            nc.scalar.activation(esq[:], sq[:], Act.Exp)
            t1 = p_row.tile([1, N], f32, tag="t1")
            nc.vector.tensor_tensor(out=t1[:], in0=esq[:], in1=inv_row[:],
                                    op=Alu.mult)
            d2_row = p_row.tile([1, N], bf16, tag="d2r")
            nc.vector.tensor_single_scalar(d2_row[:], t1[:], 2.0, Alu.mult)

            # broadcast inv and d2 across partitions via k=1 matmul
            invb_ps = p_ps.tile([128, N], f32, tag="big")
            nc.tensor.matmul(invb_ps[:], ones_row[:], inv_row[:],
                             start=True, stop=True)
            inv_bc = p_bc.tile([128, N], bf16, tag="invbc")
            nc.vector.tensor_copy(out=inv_bc[:], in_=invb_ps[:])

            d2b_ps = p_ps.tile([128, N], f32, tag="big")
            nc.tensor.matmul(d2b_ps[:], ones_row[:], d2_row[:],
                             start=True, stop=True)
            d2_bc = p_bc.tile([128, N], bf16, tag="d2bc")
            nc.vector.tensor_copy(out=d2_bc[:], in_=d2b_ps[:])

            # srt[m, n] = PU[m, n] * inv[n]   (symmetry: PU[m,n]=PU[n,m])
            srts = []
            for mc in range(4):
                srt = p_srt.tile([128, N], bf16, tag="srt")
                nc.vector.tensor_tensor(out=srt[:], in0=pus[mc][:],
                                        in1=inv_bc[:], op=Alu.mult)
                srts.append(srt)

            # ---------------- e1T: xg1^T and xg1d^T ----------------
            for pr in range(16):  # b-pairs
                b0 = 2 * pr
                ps1 = p_ps.tile([128, N], f32, tag="big")
                for mc in range(4):
                    lhs = xms[mc][:, b0:b0 + 2, :].rearrange("p b c -> p (b c)")
                    nc.tensor.matmul(ps1[:], lhs, srts[mc][:],
                                     start=(mc == 0), stop=(mc == 3))
                # xg1 psum->sbuf copies on DVE/Act (gpsimd cannot read PSUM);
                # xg1d = xg1 * d2 as sbuf-only TT on Pool engine
                ceng = [nc.scalar, nc.vector][pr % 2]
                if ceng is nc.scalar:
                    ceng.copy(out=stack2[0:C, b0, :], in_=ps1[0:C])
                else:
                    ceng.tensor_copy(out=stack2[0:C, b0, :], in_=ps1[0:C])
                ceng2 = [nc.vector, nc.scalar][pr % 2]
                if ceng2 is nc.scalar:
                    ceng2.copy(out=stack2[0:C, b0 + 1, :], in_=ps1[C:])
                else:
                    ceng2.tensor_copy(out=stack2[0:C, b0 + 1, :], in_=ps1[C:])
                nc.gpsimd.tensor_tensor(
                    out=stack2[C:, b0, :], in0=stack2[0:C, b0, :],
                    in1=d2_bc[0:C], op=Alu.mult)
                nc.gpsimd.tensor_tensor(
                    out=stack2[C:, b0 + 1, :], in0=stack2[0:C, b0 + 1, :],
                    in1=d2_bc[0:C], op=Alu.mult)

            # ---------------- per-n contraction ----------------
            for hc in range(8):  # 64-n chunks
                w0t = p_w0.tile([C + 1, 64, O], bf16, tag="w0", name=f"w0t_{t}_{hc}")
                w12t = p_w12.tile([2 * C, 64, O], f8e3, tag="w12", name=f"w12t_{t}_{hc}")
                nc.scalar.dma_start(out=w0t, in_=w0b_d[t, :, ts(hc, 64), :])
                nc.gpsimd.dma_start(out=w12t, in_=w12_d[t, :, ts(hc, 64), :])
                out_sb = p_ob.tile([B, 64, O], bf16, tag="osb")
                PG = 8
                for g in range(8):  # 8-n psum groups
                    n0 = g * PG
                    ps_o = p_pso.tile([B, PG, O], f32, tag="po")
                    for j in range(PG):
                        nl = n0 + j
                        ng = hc * 64 + nl
                        nc.tensor.matmul(
                            ps_o[:, j, :], stack1[:, :, ng],
                            w0t[:, nl, :], start=True, stop=False)
                        nc.tensor.matmul(
                            ps_o[:, j, :], stack2[:, :, ng],
                            w12t[:, nl, :], start=False, stop=True)
                    dst = out_sb[:, n0:n0 + PG, :].rearrange("b n o -> b (n o)")
                    srcp = ps_o[:].rearrange("b n o -> b (n o)")
                    oeng = [nc.vector, nc.scalar, nc.vector][g % 3]
                    if oeng is nc.scalar:
                        oeng.copy(out=dst, in_=srcp)
                    else:
                        oeng.tensor_copy(out=dst, in_=srcp)
                nc.sync.dma_start(
                    out=o_d[:, t, ts(hc, 64), :], in_=out_sb[:])

    nc.finalize()
    _CACHE[("nc", reps)] = nc
    return nc


def make_in_maps(inputs):
    import ml_dtypes
    bf16 = ml_dtypes.bfloat16
    f8e3 = ml_dtypes.float8_e3m4

    x = np.asarray(inputs["x"], dtype=np.float32)
    emb = np.asarray(inputs["dn_embeddings"], dtype=np.float32)
    w = np.asarray(inputs["weights_pool"], dtype=np.float32)
    bias = np.asarray(inputs["bias_pool"], dtype=np.float32)

    in_maps = []
    for c in range(NCORES):
        sl = slice(c * T_LOC, (c + 1) * T_LOC)
        xs = x[:, sl]  # [B, T_LOC, N, C]
        ws = w[sl]  # [T_LOC, N, K, C, O]
        # w0b rows 0:64 = (W0 - W2)^T(i,n,o), row 64 = bias
        w0b = np.empty((T_LOC, C + 1, N, O), np.float32)
        w0b[:, 0:C] = (ws[:, :, 0] - ws[:, :, 2]).transpose(0, 2, 1, 3)
        w0b[:, C] = bias[sl]
        # w12 rows 0:64 = W1^T, rows 64:128 = W2^T
        w12 = np.concatenate([
            ws[:, :, 1].transpose(0, 2, 1, 3),
            ws[:, :, 2].transpose(0, 2, 1, 3)], axis=1)
        in_maps.append({
            "xmbc_sh": np.ascontiguousarray(
                xs.transpose(1, 2, 0, 3)).astype(bf16),
            "xT_sh": np.ascontiguousarray(
                xs.transpose(1, 3, 0, 2)).astype(bf16),
            "emb_sh": np.ascontiguousarray(emb[sl]),
            "w0b_sh": np.ascontiguousarray(w0b).astype(bf16),
            "w12_sh": np.ascontiguousarray(w12).astype(f8e3),
            "ones_sh": np.ones((1, B * N), dtype=bf16),
        })
    return in_maps


def run_spmd(inputs, **kwargs):
    from concourse.bass_utils import run_bass_kernel_spmd

    nc = build_bass()
    in_maps = make_in_maps(inputs)
    res = run_bass_kernel_spmd(nc, in_maps, core_ids=list(range(NCORES)), **kwargs)
    out = np.concatenate([r["out_sh"] for r in res.results], axis=1)
    return out.astype(np.float32), res


def kernel(**inputs):
    out, _ = run_spmd(inputs)
    return out


# revision 27
# speedup vs baseline: 1.1764x; 1.0605x over previous
"""DAGCN kernel v5 for Trainium2, 8 NeuronCores, sharded over T (3 t/core).

Math per t (N=512 nodes, C=O=64, B=32, K=3):
  A    = relu(E E^T)  (rank-1 outer product, symmetric)
  PU   = exp(A) = max(exp(e_n e_m), 1)   (symmetric)
  Z_n  = sum_m PU[m, n]  (column sums == row sums by symmetry)
  S    = PU / Z  (row softmax);  d_n = S[n,n] = exp(e_n^2)/Z_n
  xg1  = S @ x;  xg2 = 2 d xg1 - x
  out  = x W0 + xg1 W1 + xg2 W2 + bias
       = x (W0 - W2) + xg1 W1 + (2 d xg1) W2 + bias     <- regrouped

Key structure vs the v3 baseline (185us):
  - Algebraic regroup removes the second message pass entirely: only
    xg1 = S@x is computed on the PE (64 vs 128 big matmuls per t), and
    xg1d = 2d*xg1 is a cheap elementwise multiply of the same psum.
  - No PE transposes for scores: PU is symmetric, so the e1T moving
    operand srt[m,n] = PU[m,n] * inv[n] is a column-scale of the PU tile.
    Column broadcasts (inv, 2d) are materialized as [128, N] tiles via a
    k=1 ones-row matmul (the PE is the only partition broadcaster).
  - Column sums via k=1 matmul with a ones column (PE, not DVE).
  - Weights quantized by numeric class: M2 weights [W1; W2] ship as
    fp8 e3m4 (they only multiply the small xg1/xg1d terms; measured
    absmax-rel ~1.3e-2 total), while [W0-W2; bias] stays bf16 (it
    multiplies x, 99.5% of output variance). Weight DMA drops from
    12.06 to 8.06 MiB/t. W0-W2 is precombined on the host.
  - Final contraction per n: 2 matmuls (k=65 bf16, k=128 with fp8
    moving), psum [32b, 8n, 64o] per group, contiguous [b, n, o] store.
"""
import sys

sys.path.insert(0, "/opt/trn_rl_repo")
import numpy as np

CFG = ""

B, T, N, C, O, K = 32, 24, 512, 64, 64, 3
NCORES = 8
T_LOC = T // NCORES  # 3 time steps per core

_CACHE = {}


def build_bass(reps=1):
    if ("nc", reps) in _CACHE:
        return _CACHE[("nc", reps)]
    from contextlib import ExitStack

    import concourse.mybir as mybir
    from concourse import bacc
    import concourse.tile as tile
    from concourse.bass import ts

    f32 = mybir.dt.float32
    f32r = mybir.dt.float32r
    bf16 = mybir.dt.bfloat16
    f8e3 = mybir.dt.float8e3
    Alu = mybir.AluOpType
    Act = mybir.ActivationFunctionType

    nc = bacc.Bacc()
    xm_d = nc.dram_tensor("xmbc_sh", [T_LOC, N, B, C], bf16, kind="ExternalInput")
    xt_d = nc.dram_tensor("xT_sh", [T_LOC, 2 * C, B // 2, N], bf16, kind="ExternalInput")
    e_d = nc.dram_tensor("emb_sh", [T_LOC, N], f32r, kind="ExternalInput")
    w0b_d = nc.dram_tensor("w0b_sh", [T_LOC, C + 1, N, O], bf16, kind="ExternalInput")
    w12_d = nc.dram_tensor("w12_sh", [T_LOC, 2 * C, N, O], f8e3, kind="ExternalInput")
    on_d = nc.dram_tensor("ones_sh", [1, B * N], bf16, kind="ExternalInput")
    # out packed for 128-partition DMA: [t, hc, g, b, h, n8, o]
    # n = hc*64 + h*32 + g*8 + n8; partitions = (g, b)
    o_d = nc.dram_tensor("out_sh", [T_LOC, 8, 128, 2, 8, O], bf16,
                         kind="ExternalOutput")

    with tile.TileContext(nc) as tc, ExitStack() as ctx:
        p1 = ctx.enter_context(tc.tile_pool(name="singles", bufs=1))
        p_row = ctx.enter_context(tc.tile_pool(name="rows", bufs=2))
        p_bc = ctx.enter_context(tc.tile_pool(name="bcast", bufs=2))
        p_pu = ctx.enter_context(tc.tile_pool(name="pu", bufs=4))
        p_srt = ctx.enter_context(tc.tile_pool(name="srt", bufs=8))
        p_x = ctx.enter_context(tc.tile_pool(name="xin", bufs=2))
        p_xs = ctx.enter_context(tc.tile_pool(name="xstage", bufs=1))
        p_w0 = ctx.enter_context(tc.tile_pool(name="w0", bufs=4))
        p_w12 = ctx.enter_context(tc.tile_pool(name="w12", bufs=4))
        p_ob = ctx.enter_context(tc.tile_pool(name="ob", bufs=2))
        p_ps = ctx.enter_context(tc.tile_pool(name="ps", bufs=4, space="PSUM"))
        p_pso = ctx.enter_context(tc.tile_pool(name="pso", bufs=2, space="PSUM"))
        p_psob = ctx.enter_context(tc.tile_pool(name="psob", bufs=2, space="PSUM"))

        # persistent stacks:
        #   stack1 [65, B, N]  rows 0:64 x^T (c,b,n), row 64 ones (bias lane)
        #   stack2 [128, B, N] rows 0:64 xg1^T, rows 64:128 xg1d^T
        stack1 = p1.tile([C + 1, B, N], bf16, name="stk1", tag="s1")
        stack2 = p1.tile([2 * C, B, N], bf16, name="stk2", tag="s2")
        # ones column [128, 1] for column-sum matmuls (k=1 trick needs a
        # [1, 128] stationary; ones row slice of on_d serves both)
        ones_col = p1.tile([128, 1], bf16, name="onec", tag="oc")
        nc.gpsimd.dma_start(out=ones_col, in_=on_d[0, 0:128].rearrange("(p f) -> p f", f=1))
        ones_row = p1.tile([1, 128], bf16, name="oner", tag="or")
        nc.gpsimd.dma_start(out=ones_row, in_=on_d[:, 0:128])

        def emit_ones_init():
            # stack1 row 64 = ones: split the [1, 32KB] single-partition DMA
            # into quarters on different queues; emitted after the critical
            # startup loads (only needed by the first final phase ~25us in)
            for q, qeng in enumerate([nc.sync, nc.scalar, nc.gpsimd, nc.sync]):
                qeng.dma_start(
                    out=stack1[C:C + 1, 8 * q:8 * (q + 1), :].rearrange(
                        "p b n -> p (b n)"),
                    in_=on_d[:, 4096 * q:4096 * (q + 1)])

        # ---- software-pipelined emission ----
        # scores(t+1) is computed during final(t): PU matmuls right after
        # e1T(t); the cheap zs/bcast matmuls interleave between final-phase
        # hc chunks so the PE queue never stalls at a dependency head.
        PG = 8

        def emit_loads(t, tt):
            st = {"t": t, "tt": tt}
            e_row = p_row.tile([1, N], f32r, tag="erow")
            nc.scalar.dma_start(out=e_row, in_=e_d[t][None, :])
            st["e_row"] = e_row
            xmall = p_x.tile([128, 4, B, C], bf16, tag="xm")
            nc.sync.dma_start(
                out=xmall,
                in_=xm_d[t].rearrange("(mc p) b c -> p mc b c", p=128))
            st["xmall"] = xmall
            xstage = p_xs.tile([2 * C, B // 2, N], bf16, tag="xst")
            nc.gpsimd.dma_start(out=xstage, in_=xt_d[t])
            st["xstage"] = xstage
            return st

        def emit_scoresA(st):
            e_row = st["e_row"]
            pus = []
            for mc in range(4):
                ps = p_ps.tile([128, N], f32, tag="big")
                nc.tensor.matmul(ps[:], e_row[:, ts(mc, 128)], e_row[:],
                                 start=True, stop=True)
                pu = p_pu.tile([128, N], bf16, tag="pu")
                nc.scalar.activation(pu[:], ps[:], Act.Exp)
                nc.vector.tensor_single_scalar(pu[:], pu[:], 1.0, Alu.max)
                pus.append(pu)
            st["pus"] = pus
            # row ops that only need e_row
            sq = p_row.tile([1, N], f32, tag="sq")
            nc.vector.tensor_mul(sq[:], e_row[:], e_row[:])
            esq = p_row.tile([1, N], f32, tag="esq")
            nc.scalar.activation(esq[:], sq[:], Act.Exp)
            st["esq"] = esq

        def emit_zs(st):
            zs_ps = p_ps.tile([128, N], f32, tag="big")
            for mc in range(4):
                nc.tensor.matmul(zs_ps[0:1, :], ones_col[:], st["pus"][mc][:],
                                 start=(mc == 0), stop=(mc == 3))
            st["zs_ps"] = zs_ps

        def emit_invd2(st):
            inv_row = p_row.tile([1, N], bf16, tag="invr")
            with nc.allow_low_precision(reason="inv feeds bf16 bcast matmul"):
                nc.vector.reciprocal(inv_row[:], st["zs_ps"][0:1, :])
            st["inv_row"] = inv_row
            t1 = p_row.tile([1, N], f32, tag="t1")
            nc.vector.tensor_tensor(out=t1[:], in0=st["esq"][:],
                                    in1=inv_row[:], op=Alu.mult)
            d2_row = p_row.tile([1, N], bf16, tag="d2r")
            nc.vector.tensor_single_scalar(d2_row[:], t1[:], 2.0, Alu.mult)
            st["d2_row"] = d2_row

        def emit_bcasts(st):
            invb_ps = p_ps.tile([128, N], f32, tag="big")
            nc.tensor.matmul(invb_ps[:], ones_row[:], st["inv_row"][:],
                             start=True, stop=True)
            inv_bc = p_bc.tile([128, N], bf16, tag="invbc")
            nc.vector.tensor_copy(out=inv_bc[:], in_=invb_ps[:])
            st["inv_bc"] = inv_bc
            d2b_ps = p_ps.tile([128, N], f32, tag="big")
            nc.tensor.matmul(d2b_ps[:], ones_row[:], st["d2_row"][:],
                             start=True, stop=True)
            d2_bc = p_bc.tile([128, N], bf16, tag="d2bc")
            nc.vector.tensor_copy(out=d2_bc[:], in_=d2b_ps[:])
            st["d2_bc"] = d2_bc

        def emit_srts(st):
            srts = []
            for mc in range(4):
                srt = p_srt.tile([128, N], bf16, tag="srt")
                nc.vector.tensor_tensor(out=srt[:], in0=st["pus"][mc][:],
                                        in1=st["inv_bc"][:], op=Alu.mult)
                srts.append(srt)
            st["srts"] = srts

        def emit_stack1_copies(st):
            xstage = st["xstage"]
            nc.vector.tensor_copy(out=stack1[0:C, 0:B // 2, :], in_=xstage[0:C])
            nc.vector.tensor_copy(out=stack1[0:C, B // 2:B, :], in_=xstage[C:])

        def emit_e1T(st):
            xmall, srts, d2_bc = st["xmall"], st["srts"], st["d2_bc"]
            for pr in range(16):  # b-pairs
                b0 = 2 * pr
                ps1 = p_ps.tile([128, N], f32, tag="big")
                for mc in range(4):
                    lhs = xmall[:, mc, b0:b0 + 2, :].rearrange("p b c -> p (b c)")
                    nc.tensor.matmul(ps1[:], lhs, srts[mc][:],
                                     start=(mc == 0), stop=(mc == 3))
                # xg1 psum->sbuf copies on DVE/Act (gpsimd cannot read PSUM);
                # xg1d = xg1 * d2 as sbuf-only TT on Pool engine
                ceng = [nc.scalar, nc.vector][pr % 2]
                if ceng is nc.scalar:
                    ceng.copy(out=stack2[0:C, b0, :], in_=ps1[0:C])
                else:
                    ceng.tensor_copy(out=stack2[0:C, b0, :], in_=ps1[0:C])
                ceng2 = [nc.vector, nc.scalar][pr % 2]
                if ceng2 is nc.scalar:
                    ceng2.copy(out=stack2[0:C, b0 + 1, :], in_=ps1[C:])
                else:
                    ceng2.tensor_copy(out=stack2[0:C, b0 + 1, :], in_=ps1[C:])
                nc.gpsimd.tensor_tensor(
                    out=stack2[C:, b0, :], in0=stack2[0:C, b0, :],
                    in1=st["d2_bc"][0:C], op=Alu.mult)
                nc.gpsimd.tensor_tensor(
                    out=stack2[C:, b0 + 1, :], in0=stack2[0:C, b0 + 1, :],
                    in1=st["d2_bc"][0:C], op=Alu.mult)

        def emit_wload(t, tt, hc):
            w0t = p_w0.tile([C + 1, 64, O], bf16, tag="w0", name=f"w0t_{tt}_{hc}")
            w12t = p_w12.tile([2 * C, 64, O], f8e3, tag="w12", name=f"w12t_{tt}_{hc}")
            nc.sync.dma_start(out=w0t, in_=w0b_d[t, :, ts(hc, 64), :])
            nc.gpsimd.dma_start(out=w12t, in_=w12_d[t, :, ts(hc, 64), :])
            return w0t, w12t

        def emit_final_chunk(t, tt, hc, w0t, w12t):
            # psum packed as 4 bands of 32 partitions; band 96 is illegal for
            # matmul (quadrant-3 HW bug): bands 0..2 in ps_a, band 3 in ps_b;
            # the copies shift band 3 to sbuf partitions 96:128.
            out_sb = p_ob.tile([128, 2, PG, O], bf16, tag="osb")
            for h in range(2):  # 32-n halves
                ps_a = p_pso.tile([96, PG, O], f32, tag="poa")
                ps_b = p_psob.tile([32, PG, O], f32, tag="pob")
                for g in range(4):  # partition bands
                    for j in range(PG):
                        nl = h * 32 + g * PG + j
                        ng = hc * 64 + nl
                        dst_ps = (ps_a[32 * g:32 * (g + 1), j, :]
                                  if g < 3 else ps_b[:, j, :])
                        nc.tensor.matmul(
                            dst_ps, stack1[:, :, ng],
                            w0t[:, nl, :], start=True, stop=False)
                        nc.tensor.matmul(
                            dst_ps, stack2[:, :, ng],
                            w12t[:, nl, :], start=False, stop=True)
                dsta = out_sb[0:96, h, :, :].rearrange("p n o -> p (n o)")
                dstb = out_sb[96:128, h, :, :].rearrange("p n o -> p (n o)")
                nc.vector.tensor_copy(
                    out=dsta, in_=ps_a[:].rearrange("p n o -> p (n o)"))
                nc.scalar.copy(
                    out=dstb, in_=ps_b[:].rearrange("p n o -> p (n o)"))
            nc.scalar.dma_start(out=o_d[t, hc], in_=out_sb[:])

        NT = T_LOC * reps
        st = emit_loads(0, 0)
        emit_scoresA(st)
        emit_ones_init()
        emit_zs(st)
        emit_invd2(st)
        emit_bcasts(st)
        emit_srts(st)
        emit_stack1_copies(st)
        for tt in range(NT):
            t = tt % T_LOC
            emit_e1T(st)
            # prefetch first 4 weight chunks before next-t input loads so the
            # final phase is never starved behind them on the queues
            wts = {hc: emit_wload(t, tt, hc) for hc in range(3)}
            nxt = None
            if tt + 1 < NT:
                nxt = emit_loads((tt + 1) % T_LOC, tt + 1)
                emit_scoresA(nxt)
            for hc in range(8):
                emit_final_chunk(t, tt, hc, *wts.pop(hc))
                if hc + 3 < 8:
                    wts[hc + 3] = emit_wload(t, tt, hc + 3)
                if nxt is not None:
                    if hc == 0:
                        emit_zs(nxt)
                    elif hc == 1:
                        emit_invd2(nxt)
                    elif hc == 3:
                        emit_bcasts(nxt)
                    elif hc == 5:
                        emit_srts(nxt)
            if nxt is not None:
                emit_stack1_copies(nxt)
            st = nxt

    nc.finalize()
    _CACHE[("nc", reps)] = nc
    return nc


def make_in_maps(inputs):
    import ml_dtypes
    bf16 = ml_dtypes.bfloat16
    f8e3 = ml_dtypes.float8_e3m4

    x = np.asarray(inputs["x"], dtype=np.float32)
    emb = np.asarray(inputs["dn_embeddings"], dtype=np.float32)
    w = np.asarray(inputs["weights_pool"], dtype=np.float32)
    bias = np.asarray(inputs["bias_pool"], dtype=np.float32)

    in_maps = []
    for c in range(NCORES):
        sl = slice(c * T_LOC, (c + 1) * T_LOC)
        xs = x[:, sl]  # [B, T_LOC, N, C]
        ws = w[sl]  # [T_LOC, N, K, C, O]
        # w0b rows 0:64 = (W0 - W2)^T(i,n,o), row 64 = bias
        w0b = np.empty((T_LOC, C + 1, N, O), np.float32)
        w0b[:, 0:C] = (ws[:, :, 0] - ws[:, :, 2]).transpose(0, 2, 1, 3)
        w0b[:, C] = bias[sl]
        # w12 rows 0:64 = W1^T, rows 64:128 = W2^T
        w12 = np.concatenate([
            ws[:, :, 1].transpose(0, 2, 1, 3),
            ws[:, :, 2].transpose(0, 2, 1, 3)], axis=1)
        xT = xs.transpose(1, 3, 0, 2)  # [T_LOC, C, B, N]
        xT2 = np.concatenate([xT[:, :, :B // 2], xT[:, :, B // 2:]], axis=1)
        in_maps.append({
            "xmbc_sh": np.ascontiguousarray(
                xs.transpose(1, 2, 0, 3)).astype(bf16),
            "xT_sh": np.ascontiguousarray(xT2).astype(bf16),
            "emb_sh": np.ascontiguousarray(emb[sl]),
            "w0b_sh": np.ascontiguousarray(w0b).astype(bf16),
            "w12_sh": np.ascontiguousarray(w12).astype(f8e3),
            "ones_sh": np.ones((1, B * N), dtype=bf16),
        })
    return in_maps


def run_spmd(inputs, **kwargs):
    from concourse.bass_utils import run_bass_kernel_spmd

    nc = build_bass()
    in_maps = make_in_maps(inputs)
    res = run_bass_kernel_spmd(nc, in_maps, core_ids=list(range(NCORES)), **kwargs)
    outs = []
    for r in res.results:
        o2 = np.asarray(r["out_sh"]).reshape(T_LOC, 8, 4, B, 2, 8, O)
        # n = hc*64 + h*32 + g*8 + n8  ->  [B, T_LOC, hc, h, g, n8, O]
        o = o2.transpose(3, 0, 1, 4, 2, 5, 6).reshape(B, T_LOC, N, O)
        outs.append(o)
    out = np.concatenate(outs, axis=1)
    return out.astype(np.float32), res


def kernel(**inputs):
    out, _ = run_spmd(inputs)
    return out
